# revision 10
# baseline (speedup 1.0000x reference)
"""Trainium2 Bass kernel for dist-biased multi-head attention.

Reference computation (jax):
    qkv = x @ w_qkv; q,k,v = split(qkv); heads of 64
    dots = einsum('bhnd,bhmd->bhnm', q, k) * scale + dist
    attn = softmax(dots, axis=-1)
    out  = einsum('bhnm,bhmd->bhnd', attn, v) -> merge heads -> @ w_out + b_out

Shapes: x [4, 2048, 512], dist [4, 8, 2048, 2048], w_qkv [512, 1536],
w_out [512, 512], b_out [512].

Sharding over 8 cores: core m handles batch m//2, heads 4*(m%2) .. +4.
Each core computes its 4 heads' attention plus the partial out-projection
for its batch; host sums the two partials per batch and adds b_out.

v2 design (ACT-bound pipeline, target ~150-165us/core):
 - scores computed TRANSPOSED: S^T [keys(part), queries(free)], so AV
   contracts keys on partitions with no transposes. Head PAIRS stacked on
   partitions for q/k (rows 0-63 = head 2p, 64-127 = head 2p+1).
 - dist folded in as exp(dots) = exp(qk)*exp(dist): host ships exp(dist^T)
   bf16 packed [pair, kb, row128, s, n]; one [128, 2x1024] DVE multiply per
   (pair, kb) applies it (2x bf16 mode).  Optional "mixN" variants put N
   evenly-spaced key blocks on the PE instead (raw dist via identity-matmul
   accumulate) to rebalance DVE vs PE.
 - ACT does ONLY the 128 exp activations in steady state ([128,1024] each,
   ~1147ns) - that is the binding engine at ~147us. All evacuation copies
   live on DVE (or ACT only in phase 1 / tail where ACT is idle).
 - softmax denominator: v augmented with a ones column at COLUMN 0, so the
   denominator lands on PSUM partition 0 and reciprocal_approx_fast reads
   it in place (base-partition-0 requirement).  Normalization is fused:
   oT2 = po[1:65] * gpsimd-broadcast(recip), read straight from PSUM.
 - kb loop software-pipelined: AV(kb) is emitted after QK(kb+1)/exp(kb+1)
   so the PE never head-of-line-blocks the exp pipeline.
 - PSUM budget exactly 8 banks: ps ring 2x[128,1024] (4) + po 2x[65,1024]
   (4); out-projection and the deferred q-half1 projection borrow ps slots.
"""

import numpy as np

N_CORES = 8
B = 4
NTOK = 2048
DIM = 512
HEADS = 8
DH = 64  # head dim
NH = HEADS // 2  # heads per core (4)
NPAIR = NH // 2  # head pairs per core (2)
INNER = HEADS * DH
SCALE = DH ** -0.5
QC = 1024  # query chunk (free-dim) per attention psum block
NKB = NTOK // 128  # key blocks of 128


def _pe_kb(variant):
    """Key blocks whose dist-add runs on the PE (identity-matmul accumulate)."""
    import re

    m = re.search(r"mix(\d+)", variant)
    if not m:
        return ()
    x = int(m.group(1))
    return tuple(round((i + 0.5) * NKB / x) for i in range(x))


def _build_nc(repeats=1, variant="v2"):
    """variant flags (substring match):
      v2    - default: all dist via DVE exp-multiply
      mixN  - N evenly-spaced key blocks moved to PE identity-add
      timing-only ablations (results wrong): nodma, nomul, noav.
    """
    import concourse.bacc as bacc
    import concourse.mybir as mybir
    import concourse.tile as tile
    from concourse.bass import ts
    from concourse.masks import make_identity

    f32 = mybir.dt.float32
    bf16 = mybir.dt.bfloat16
    Exp = mybir.ActivationFunctionType.Exp

    pe_kb = _pe_kb(variant)

    nc = bacc.Bacc("TRN2", target_bir_lowering=False, debug=False)

    xT_d = nc.dram_tensor("xT", [DIM, NTOK], bf16, kind="ExternalInput").ap()
    wq_d = nc.dram_tensor("wq", [DIM, NH * DH], bf16, kind="ExternalInput").ap()
    wk_d = nc.dram_tensor("wk", [DIM, NH * DH], bf16, kind="ExternalInput").ap()
    wv_d = nc.dram_tensor("wv", [DIM, NH * DH], bf16, kind="ExternalInput").ap()
    # exp(dist^T) (or raw dist^T for PE blocks), packed [pair, kb, row, s, n]
    ed_d = nc.dram_tensor(
        "expdT", [NPAIR, NKB, 128, 2, NTOK], bf16, kind="ExternalInput"
    ).ap()
    wo_d = nc.dram_tensor("wo", [NH * DH, DIM], bf16, kind="ExternalInput").ap()
    part_d = nc.dram_tensor("part", [NTOK, DIM], f32, kind="ExternalOutput").ap()

    with tile.TileContext(nc) as tc:
        for _rep in range(repeats):
            with (
                tc.tile_pool(name="consts", bufs=1) as consts,
                tc.tile_pool(name="qkv", bufs=1) as qkv,
            ):
                # weights first (small, unblock k-proj), then xT per c-chunk
                # so the first projection starts after ~0.5MB, not 2.75MB
                wk_sb = consts.tile([128, DIM // 128, NH * DH], bf16)
                nc.sync.dma_start(wk_sb[:], wk_d.rearrange("(c p) n -> p c n", p=128))
                wq_sb = consts.tile([128, DIM // 128, NH * DH], bf16)
                nc.sync.dma_start(wq_sb[:], wq_d.rearrange("(c p) n -> p c n", p=128))
                wv_sb = consts.tile([128, DIM // 128, NH * DH], bf16)
                nc.sync.dma_start(wv_sb[:], wv_d.rearrange("(c p) n -> p c n", p=128))
                # w_out rows for the pair stacked on partitions: [128, pair, DIM]
                wo_sb = consts.tile([128, NPAIR, DIM], bf16)
                nc.sync.dma_start(wo_sb[:], wo_d.rearrange("(p q) n -> q p n", q=128))
                xT_sb = consts.tile([128, DIM // 128, NTOK], bf16)
                xT_r = xT_d.rearrange("(c p) n -> p c n", p=128)
                for c in range(DIM // 128):
                    nc.sync.dma_start(xT_sb[:, c, :], xT_r[:, c, :])

                if pe_kb:
                    ident32 = consts.tile([128, 128], f32)
                    make_identity(nc, ident32)
                    ident = consts.tile([128, 128], bf16)
                    nc.scalar.copy(ident[:], ident32[:])

                # head pair p stacked on partitions: rows 0-63 head 2p, 64-127 head 2p+1
                qT2 = qkv.tile([128, NPAIR, NTOK], bf16)
                kT2 = qkv.tile([128, NPAIR, NTOK], bf16)
                # v with a ones column at col DH: AV output row 64 = softmax denom
                v_sb = qkv.tile([128, NH, NKB, DH + 1], bf16)
                ones32 = consts.tile([128, NH, NKB, 1], f32)
                nc.gpsimd.memset(ones32[:], 1.0)
                nc.scalar.copy(v_sb[:, :, :, DH : DH + 1], ones32[:])

                # ---- phase 1: projections (k, v, q-half0; q-half1 is
                # interleaved into qc0's attention loop) ----
                with (
                    tc.tile_pool(name="p1qk", bufs=2, space="PSUM") as p1qk,
                    tc.tile_pool(name="p1v", bufs=2, space="PSUM") as p1v,
                ):
                    def qk_proj(dst, w_sb, p, half, pool, tag=""):
                        kw = {"tag": tag} if tag else {}
                        ps = pool.tile([128, QC], f32, name="ps", **kw)
                        for c in range(DIM // 128):
                            for j in range(QC // 512):
                                nc.tensor.matmul(
                                    ps[:, ts(j, 512)],
                                    w_sb[:, c, ts(p, 128)],
                                    xT_sb[:, c, half * QC + 512 * j : half * QC + 512 * (j + 1)],
                                    start=(c == 0),
                                    stop=(c == DIM // 128 - 1),
                                )
                        if tag:
                            # interleaved into attention: ACT is saturated there
                            nc.vector.tensor_copy(dst[:, p, ts(half, QC)], ps[:])
                        else:
                            nc.scalar.copy(dst[:, p, ts(half, QC)], ps[:])

                    # order so attention (p=0, qc=0) can start earliest
                    qk_proj(kT2, wk_sb, 0, 0, p1qk)
                    qk_proj(kT2, wk_sb, 0, 1, p1qk)
                    qk_proj(qT2, wq_sb, 0, 0, p1qk)
                    for i in range(NKB):
                        ps_v = p1v.tile([128, NH * DH], f32)
                        for c in range(DIM // 128):
                            nc.tensor.matmul(
                                ps_v[:],
                                xT_sb[:, c, ts(i, 128)],
                                wv_sb[:, c, :],
                                start=(c == 0),
                                stop=(c == DIM // 128 - 1),
                            )
                        nc.scalar.copy(
                            v_sb[:, :, i, 0:DH],
                            ps_v.rearrange("p (h d) -> p h d", h=NH),
                        )
                    qk_proj(kT2, wk_sb, 1, 0, p1qk)
                    qk_proj(kT2, wk_sb, 1, 1, p1qk)
                    qk_proj(qT2, wq_sb, 1, 0, p1qk)

                # ---- phase 2+3: attention + out-projection ----
                with (
                    tc.tile_pool(name="spsum", bufs=2, space="PSUM") as spsum,
                    tc.tile_pool(name="opsum", bufs=2, space="PSUM") as opsum,
                    tc.tile_pool(name="distp", bufs=14) as distp,
                    tc.tile_pool(name="expp", bufs=4) as expp,
                    tc.tile_pool(name="emp", bufs=4) as emp,
                    tc.tile_pool(name="otp", bufs=4) as otp,
                    tc.tile_pool(name="smalls", bufs=2) as smalls,
                    tc.tile_pool(name="outp", bufs=3) as outp,
                ):
                    def out_proj(oT2_src, qc_src, i, act_evac=False):
                        # pair-stacked K=128, accumulate pairs in PSUM
                        pp = spsum.tile([128, DIM], f32, name="pp", tag="ps")
                        for p2 in range(NPAIR):
                            nc.tensor.matmul(
                                pp[:],
                                oT2_src[p2][:, ts(i, 128)],
                                wo_sb[:, p2, :],
                                start=(p2 == 0),
                                stop=(p2 == NPAIR - 1),
                            )
                        ob = outp.tile([128, DIM], f32, name="ob")
                        if act_evac:
                            nc.scalar.copy(ob[:], pp[:])
                        else:
                            nc.vector.tensor_copy(ob[:], pp[:])
                        # out DMA on the gpsimd queue: keeps the sync queue
                        # free for the ed prefetch stream (no HOL blocking)
                        nc.gpsimd.dma_start(
                            part_d[qc_src * QC + i * 128 : qc_src * QC + (i + 1) * 128, :],
                            ob[:],
                        )

                    prev_oT2 = None
                    for qc in range(NTOK // QC):
                        oT2 = [otp.tile([128, QC], bf16, name="oT2") for _ in range(NPAIR)]
                        for p in range(NPAIR):
                            po = [opsum.tile([DH + 1, QC], f32, name="po") for _ in range(2)]
                            # software pipeline: AV for block kb runs one
                            # iteration behind QK/exp so QK(kb+1) precedes
                            # AV(kb) in the PE queue (no HOL blocking of exp)
                            pend = None

                            def do_av(em_src, kb_src):
                                for s, j in ((0, 0), (1, 0), (0, 1), (1, 1)):
                                    nc.tensor.matmul(
                                        po[s][:, ts(j, 512)],
                                        v_sb[:, 2 * p + s, kb_src, :],
                                        em_src[:, s, ts(j, 512)],
                                        start=(kb_src == 0),
                                        stop=(kb_src == NKB - 1),
                                    )

                            for kb in range(NKB):
                                ed2 = distp.tile([128, 2, QC], bf16, name="ed")
                                if "nodma" not in variant:
                                    nc.sync.dma_start(
                                        ed2[:], ed_d[p, kb, :, :, ts(qc, QC)]
                                    )
                                on_pe = kb in pe_kb and "nomul" not in variant
                                ps = [
                                    spsum.tile([128, QC], f32, name="ps", tag="ps")
                                    for _ in range(2)
                                ]
                                # two K=64 row-tiles (rows 0-63 / 64-127),
                                # issued alternating so adjacent MMs target
                                # disjoint row groups
                                for s, j in ((0, 0), (1, 0), (0, 1), (1, 1)):
                                    pb = 64 * s
                                    nc.tensor.matmul(
                                        ps[s][:, ts(j, 512)],
                                        kT2[pb : pb + 64, p, ts(kb, 128)],
                                        qT2[pb : pb + 64, p, qc * QC + 512 * j : qc * QC + 512 * (j + 1)],
                                        start=True,
                                        stop=not on_pe,
                                    )
                                if on_pe:
                                    for s, j in ((0, 0), (1, 0), (0, 1), (1, 1)):
                                        nc.tensor.matmul(
                                            ps[s][:, ts(j, 512)],
                                            ident[:],
                                            ed2[:, s, ts(j, 512)],
                                            start=False,
                                            stop=True,
                                        )
                                ex2 = expp.tile([128, 2, QC], bf16, name="ex")
                                for s in range(2):
                                    nc.scalar.activation(ex2[:, s, :], ps[s][:], Exp)
                                if on_pe or "nomul" in variant:
                                    em2 = ex2
                                else:
                                    em2 = emp.tile([128, 2, QC], bf16, name="em")
                                    nc.vector.tensor_mul(em2[:], ex2[:], ed2[:])
                                if pend is not None and "noav" not in variant:
                                    do_av(*pend)
                                pend = (em2, kb)
                                # previous chunk's out-projection: grouped 4
                                # slabs per pair right after kb1 — an EVEN
                                # number of ps-tag borrows keeps the s0/s1
                                # ring parity intact for the rest of the pass
                                if prev_oT2 is not None and kb == 1:
                                    for i4 in range(4):
                                        out_proj(prev_oT2, qc - 1, 4 * p + i4)
                                # qc0: deferred q-half1 projections, adjacent
                                # pair of borrows (parity-preserving)
                                if qc == 0 and p == 0 and kb == 3:
                                    qk_proj(qT2, wq_sb, 0, 1, spsum, tag="ps")
                                    qk_proj(qT2, wq_sb, 1, 1, spsum, tag="ps")
                            if pend is not None and "noav" not in variant:
                                do_av(*pend)
                            # normalize (softmax denom = po row 64): evacuate
                            # po quickly (den+poc copies, ~2.2us) so the next
                            # pair's AV gets the PSUM banks; recip/broadcast/
                            # scale then run from SBUF off the critical path.
                            # reciprocal_approx_fast needs a base-partition-0
                            # input (base-64 views return garbage) => den copy
                            for s in range(2):
                                den = smalls.tile([1, QC], f32, name="den", tag="den")
                                nc.vector.tensor_copy(den[:], po[s][DH : DH + 1, :])
                                poc = smalls.tile([DH, QC], f32, name="poc", tag="poc")
                                nc.vector.tensor_copy(poc[:], po[s][0:DH, :])
                                rec = smalls.tile([1, QC], f32, name="rec", tag="rec")
                                nc.vector.reciprocal_approx_fast(rec[:], den[:])
                                rb = smalls.tile([DH, QC], f32, name="rb", tag="rb")
                                nc.gpsimd.partition_broadcast(rb[:], rec[:])
                                nc.vector.tensor_mul(
                                    oT2[p][64 * s : 64 * s + 64, :],
                                    poc[:],
                                    rb[:],
                                )
                        prev_oT2 = oT2
                    # last chunk's out-projection (tail): alternate evac
                    # engines so the 8 units pipeline instead of serializing
                    for i in range(QC // 128):
                        out_proj(prev_oT2, NTOK // QC - 1, i, act_evac=(i % 2 == 0))

    nc.compile()
    return nc


_NC_CACHE = {}


def _get_nc(repeats=1, variant=None):
    if variant is None:
        variant = KERNEL_VARIANT
    key = (repeats, variant)
    if key not in _NC_CACHE:
        _NC_CACHE[key] = _build_nc(repeats, variant)
    return _NC_CACHE[key]


def make_in_maps(x, dist, w_qkv, w_out, variant=None):
    """Host-side sharding: per-core input dicts (final device dtypes)."""
    import ml_dtypes

    if variant is None:
        variant = KERNEL_VARIANT
    bf16 = ml_dtypes.bfloat16
    pe_kb = _pe_kb(variant)
    x = np.asarray(x, dtype=np.float32)
    dist = np.asarray(dist, dtype=np.float32)
    w_qkv = np.asarray(w_qkv, dtype=np.float32)
    w_out = np.asarray(w_out, dtype=np.float32)
    in_maps = []
    for m in range(N_CORES):
        b = m // 2
        h0 = NH * (m % 2)
        wq = np.ascontiguousarray(w_qkv[:, h0 * DH : (h0 + NH) * DH]) * np.float32(SCALE)
        wk = np.ascontiguousarray(w_qkv[:, INNER + h0 * DH : INNER + (h0 + NH) * DH])
        wv = np.ascontiguousarray(w_qkv[:, 2 * INNER + h0 * DH : 2 * INNER + (h0 + NH) * DH])
        dT = np.ascontiguousarray(dist[b, h0 : h0 + NH].transpose(0, 2, 1))
        ed = np.exp(dT)
        for kb in pe_kb:
            # PE blocks carry raw dist (identity-add on the tensor engine)
            ed[:, kb * 128 : (kb + 1) * 128, :] = dT[:, kb * 128 : (kb + 1) * 128, :]
        # pack [NH, keys, n] -> [pair, kb, row, s, n]
        ed5 = np.ascontiguousarray(
            ed.reshape(NPAIR, 2, NKB, 128, NTOK).transpose(0, 2, 3, 1, 4)
        )
        in_maps.append(
            {
                "xT": np.ascontiguousarray(x[b].T).astype(bf16),
                "wq": wq.astype(bf16),
                "wk": wk.astype(bf16),
                "wv": wv.astype(bf16),
                "expdT": ed5.astype(bf16),
                "wo": np.ascontiguousarray(w_out[h0 * DH : (h0 + NH) * DH, :]).astype(bf16),
            }
        )
    return in_maps


def cast_in_maps(nc, in_maps):
    """Compat shim: make_in_maps already emits final dtypes."""
    return in_maps


def assemble(results, b_out):
    """Sum the two per-batch partials and add bias."""
    out = np.empty((B, NTOK, DIM), dtype=np.float32)
    for b in range(B):
        out[b] = results[2 * b]["part"] + results[2 * b + 1]["part"] + b_out
    return out


KERNEL_VARIANT = "v2"


def kernel(x, dist, w_qkv, w_out, b_out):
    from concourse.bass_utils import run_bass_kernel_spmd

    nc = _get_nc(variant=KERNEL_VARIANT)
    in_maps = make_in_maps(x, dist, w_qkv, w_out)
    res = run_bass_kernel_spmd(nc, in_maps, core_ids=list(range(N_CORES)))
    return assemble(res.results, np.asarray(b_out, dtype=np.float32))
